# revision 1
# baseline (speedup 1.0000x reference)
"""BERT+CRF loss (torchcrf-style, reduction=sum) on 8 Trainium2 NeuronCores.

Strategy (pure data parallel, batch sharded 8 ways, 8 sequences per core):
  emissions^T = W^T @ X^T on TensorE (X pre-transposed on host, f32)
  CRF forward recurrence in exp space:
      v_t = (v_{t-1}^T expT) * E_t,  E_t = exp(em_t)
  Adjacent steps are paired into 9x9 transfer matrices
      B_p[i,j] = sum_k expT[i,k] E_{2p+1}[k] expT[k,j] E_{2p+2}[j]
  computed on TensorE as  outer(E_a, E_b) [81] x G4 [81,81]  (G4 is a host
  constant built from exp(trans)).  Each sequence's 255 pair matrices are
  split into 16 chunks of 16; a chunk-parallel matrix product runs on
  VectorE with 128 partitions = 8 batches x 16 chunks, 16 steps, periodic
  max-normalization for range safety.  Host combines the 16 chunk matrices
  per sequence (O(B*16*81) f64) and adds the label-indexed numerator terms.
"""

import sys

if "/opt/trn_rl_repo" not in sys.path:
    sys.path.insert(0, "/opt/trn_rl_repo")

import numpy as np

B, S, H, L = 64, 512, 768, 9
NCORES = 8
BPC = B // NCORES          # sequences per core
LL = L * L                 # 81
NPAIR = 256                # pair slots per sequence (255 real + 1 identity)
NCHUNK = 16                # chunks per sequence
SPC = NPAIR // NCHUNK      # pair-steps per chunk = 16
HC = H // 128              # 6 contraction chunks of 128
NORM_STEPS = (5, 11, 15)   # recurrence steps after which we renormalize
NNORM = len(NORM_STEPS)

_CACHE = {}


def _build_bass():
    import concourse.bass as bass
    import concourse.bacc as bacc
    import concourse.mybir as mybir
    import concourse.tile as tile
    from contextlib import ExitStack

    f32 = mybir.dt.float32
    bf16 = mybir.dt.bfloat16
    Alu = mybir.AluOpType
    Act = mybir.ActivationFunctionType
    Ax = mybir.AxisListType

    nc = bacc.Bacc()

    # ---- I/O ----
    xT_d = nc.dram_tensor("xT", [BPC, H, S], f32, kind="ExternalInput")
    w_d = nc.dram_tensor("Wt", [H, L], f32, kind="ExternalInput")
    lab_d = nc.dram_tensor("lab9", [BPC, L, S], f32, kind="ExternalInput")
    g4_d = nc.dram_tensor("G4", [LL, LL], f32, kind="ExternalInput")
    ra_d = nc.dram_tensor("Ra", [L, LL], f32, kind="ExternalInput")
    rb_d = nc.dram_tensor("Rb", [L, LL], f32, kind="ExternalInput")
    iota_d = nc.dram_tensor("iota9", [L, 1], f32, kind="ExternalInput")
    id_d = nc.dram_tensor("id128", [128, LL], f32, kind="ExternalInput")

    s_out = nc.dram_tensor("S_out", [128, LL], f32, kind="ExternalOutput")
    m_out = nc.dram_tensor("m_out", [128, NNORM], f32, kind="ExternalOutput")
    e_out = nc.dram_tensor("e_out", [BPC, L, 2], f32, kind="ExternalOutput")
    nt_out = nc.dram_tensor("nt_out", [L, BPC], f32, kind="ExternalOutput")

    with ExitStack() as ctx:
        tc = ctx.enter_context(tile.TileContext(nc))
        const = ctx.enter_context(tc.tile_pool(name="const", bufs=1))
        xpool = ctx.enter_context(tc.tile_pool(name="x", bufs=3))
        xbpool = ctx.enter_context(tc.tile_pool(name="xb", bufs=2))
        epool = ctx.enter_context(tc.tile_pool(name="e", bufs=2))
        lpool = ctx.enter_context(tc.tile_pool(name="lab", bufs=3))
        spool = ctx.enter_context(tc.tile_pool(name="sm", bufs=3))
        rpool = ctx.enter_context(tc.tile_pool(name="rec", bufs=1))
        dpool = ctx.enter_context(tc.tile_pool(name="dram", bufs=1, space="DRAM"))
        ps_em = ctx.enter_context(tc.tile_pool(name="psem", bufs=3, space="PSUM"))
        ps_rep = ctx.enter_context(tc.tile_pool(name="psrep", bufs=1, space="PSUM"))
        ps_b = ctx.enter_context(tc.tile_pool(name="psb", bufs=2, space="PSUM"))

        # ---- constants into SBUF (matmul operands cast to bf16 by DMA) ----
        w_sb = const.tile([128, HC, L], bf16)
        nc.gpsimd.dma_start(w_sb[:], w_d[:].rearrange("(c k) l -> k c l", c=HC))
        g4_sb = const.tile([LL, LL], bf16)
        nc.gpsimd.dma_start(g4_sb[:], g4_d[:])
        ra_sb = const.tile([L, LL], bf16)
        nc.gpsimd.dma_start(ra_sb[:], ra_d[:])
        rb_sb = const.tile([L, LL], bf16)
        nc.gpsimd.dma_start(rb_sb[:], rb_d[:])
        iota_sb = const.tile([L, 1], f32)
        nc.sync.dma_start(iota_sb[:], iota_d[:])

        # ---- persistent recurrence state ----
        s_tile = rpool.tile([128, LL], f32)            # chunk-product state
        nc.sync.dma_start(s_tile[:], id_d[:])          # init to I (per row)
        bc_tile = rpool.tile([128, SPC * LL], f32)     # pair matrices, chunk layout
        tmp729 = rpool.tile([128, L * L * L], f32)
        mvals = rpool.tile([128, NNORM], f32)
        emtag = rpool.tile([L, BPC], f32)

        # internal DRAM bounce for pair matrices; row 255 of each b = identity
        b_all = dpool.tile([BPC, NPAIR, LL], f32)
        for b in range(BPC):
            nc.scalar.dma_start(b_all[b, NPAIR - 1, :], id_d[0, :])

        for b in range(BPC):
            # stream X^T for this sequence (f32, HWDGE, two queue-spread DMAs),
            # then cast to bf16 on the otherwise-idle GpSimd engine
            xt = xpool.tile([128, HC, S], f32)
            src = xT_d[b].rearrange("(c k) s -> k c s", c=HC)
            nc.sync.dma_start(xt[:, 0 : HC // 2, :], src[:, 0 : HC // 2, :])
            nc.sync.dma_start(xt[:, HC // 2 : HC, :], src[:, HC // 2 : HC, :])
            xtb = xbpool.tile([128, HC, S], bf16)
            nc.vector.tensor_copy(xtb[:], xt[:])

            # emissions^T [9, S] in PSUM (no bias; handled on host)
            em_ps = ps_em.tile([L, S], f32)
            for c in range(HC):
                nc.tensor.matmul(
                    em_ps[:], w_sb[:, c, :], xtb[:, c, :],
                    start=(c == 0), stop=(c == HC - 1),
                )

            # E = exp(em) in bf16, with one extra zero column at index S
            e_sb = epool.tile([L, S + 1], bf16)
            nc.vector.memset(e_sb[:, S : S + 1], 0.0)
            nc.scalar.activation(e_sb[:, 0:S], em_ps[:], Act.Exp)
            # export exp of em columns 0 and S-1 in f32 for host (v0, tail)
            em01 = bass.AP(
                em_ps.tensor, em_ps[:].offset, [[em_ps[:].ap[0][0], L], [S - 1, 2]]
            )
            e01 = spool.tile([L, 2], f32)
            nc.scalar.activation(e01[:], em01, Act.Exp)
            nc.sync.dma_start(e_out[b], e01[:])

            # numerator: sum_t em[label_t, t] accumulated per (l, b)
            lb = lpool.tile([L, S], f32)
            nc.scalar.dma_start(lb[:], lab_d[b])
            msk = spool.tile([L, S], f32)
            nc.vector.scalar_tensor_tensor(
                out=msk[:], in0=lb[:], scalar=iota_sb[:], in1=em_ps[:],
                op0=Alu.is_equal, op1=Alu.mult,
                accum_out=emtag[:, b : b + 1],
            )

            # replicate E_odd / E_even into [81, 256] via TensorE
            ap0 = e_sb[:].ap[0]
            ea_ap = bass.AP(e_sb.tensor, e_sb[:].offset + 1, [[ap0[0], L], [2, NPAIR]])
            eb_ap = bass.AP(e_sb.tensor, e_sb[:].offset + 2, [[ap0[0], L], [2, NPAIR]])
            earep = ps_rep.tile([LL, NPAIR], f32)
            nc.tensor.matmul(earep[:], ra_sb[:], ea_ap, start=True, stop=True)
            ebrep = ps_rep.tile([LL, NPAIR], f32)
            nc.tensor.matmul(ebrep[:], rb_sb[:], eb_ap, start=True, stop=True)
            eacp = spool.tile([LL, NPAIR], bf16)
            nc.scalar.copy(eacp[:], earep[:])
            ebcp = spool.tile([LL, NPAIR], bf16)
            nc.scalar.copy(ebcp[:], ebrep[:])
            outer = spool.tile([LL, NPAIR], bf16)
            nc.vector.tensor_mul(outer[:], eacp[:], ebcp[:])

            # pair matrices B_p = outer^T @ G4, two halves of 128 pairs
            for h in range(2):
                bp = ps_b.tile([128, LL], f32)
                nc.tensor.matmul(
                    bp[:], outer[:, h * 128 : (h + 1) * 128], g4_sb[:],
                    start=True, stop=True,
                )
                bsb = spool.tile([128, LL], f32)
                nc.scalar.copy(bsb[:], bp[:])
                rows = 128 if h == 0 else 127   # skip pair 255 (stays identity)
                nc.sync.dma_start(
                    b_all[b, h * 128 : h * 128 + rows, :], bsb[0:rows, :]
                )
            # chunk-layout rows for this sequence: partition 16*b+c
            nc.scalar.dma_start(
                bc_tile[16 * b : 16 * (b + 1), :],
                b_all[b].rearrange("(c s) j -> c (s j)", c=NCHUNK),
            )

        # ---- chunk-parallel matrix recurrence: S <- S @ B_s ----
        ncol = 0
        for s in range(SPC):
            bs = bc_tile[:, s * LL : (s + 1) * LL]
            in0 = (
                s_tile[:].rearrange("p (i k) -> p i k", i=L)
                .unsqueeze(2).broadcast_to([128, L, L, L])
            )
            # bc stores B^T (column-major B): inner k is contiguous
            in1 = (
                bs.rearrange("p (j k) -> p j k", j=L)
                .unsqueeze(1).broadcast_to([128, L, L, L])
            )
            t3 = tmp729[:].rearrange("p (i j k) -> p i j k", i=L, j=L)
            nc.vector.tensor_tensor(out=t3, in0=in0, in1=in1, op=Alu.mult)
            nc.vector.tensor_reduce(
                out=s_tile[:], in_=t3, axis=Ax.X, op=Alu.add
            )
            if s in NORM_STEPS:
                mc = mvals[:, ncol : ncol + 1]
                ncol += 1
                nc.vector.reduce_max(mc, s_tile[:], axis=Ax.X)
                rec = spool.tile([128, 1], f32)
                nc.vector.reciprocal(rec[:], mc)
                nc.vector.tensor_scalar_mul(s_tile[:], s_tile[:], rec[:])

        nc.sync.dma_start(s_out[:], s_tile[:])
        nc.sync.dma_start(m_out[:], mvals[:])
        nc.sync.dma_start(nt_out[:], emtag[:])

    if not nc.is_finalized():
        nc.finalize()
    return nc


def _get_nc():
    if "nc" not in _CACHE:
        _CACHE["nc"] = _build_bass()
    return _CACHE["nc"]


def _host_consts(trans):
    expT = np.exp(trans.astype(np.float64)).astype(np.float32)  # [9,9]
    k_idx = np.arange(LL) // L   # row index of the 81-flat (k, jb)
    jb_idx = np.arange(LL) % L
    i_idx = np.arange(LL) // L   # col index of the 81-flat (i, j)
    j_idx = np.arange(LL) % L
    # G4[(k,jb),(i,j)] = expT[i,k] * expT[k,j] * (j == jb)
    g4 = (
        expT[np.ix_(i_idx, k_idx)].T
        * expT[np.ix_(k_idx, j_idx)]
        * (j_idx[None, :] == jb_idx[:, None])
    ).astype(np.float32)
    # store B transposed (column-major) so the recurrence reads contiguously
    g4 = np.ascontiguousarray(g4.reshape(LL, L, L).swapaxes(1, 2).reshape(LL, LL))
    ra = (k_idx[None, :] == np.arange(L)[:, None]).astype(np.float32)   # [9,81]
    rb = (jb_idx[None, :] == np.arange(L)[:, None]).astype(np.float32)  # [9,81]
    iota = np.arange(L, dtype=np.float32).reshape(L, 1)
    id128 = np.tile(np.eye(L, dtype=np.float32).reshape(1, LL), (128, 1))
    return expT, g4, ra, rb, iota, id128


def _numpy_reference(hs, mask, labels, W, bb, st, en, tr):
    # general fallback (only used when attention_mask is not all ones)
    em = hs.astype(np.float64) @ W.astype(np.float64) + bb.astype(np.float64)
    maskb = mask.astype(bool)
    maskf = mask.astype(np.float64)
    em_tag = np.take_along_axis(em, labels[..., None], axis=-1)[..., 0]
    num = st.astype(np.float64)[labels[:, 0]] + em_tag[:, 0]
    trs = tr.astype(np.float64)[labels[:, :-1], labels[:, 1:]]
    num = num + np.sum((trs + em_tag[:, 1:]) * maskf[:, 1:], axis=1)
    last = mask.sum(axis=1).astype(np.int64) - 1
    num = num + en.astype(np.float64)[labels[np.arange(len(labels)), last]]
    alpha = st.astype(np.float64)[None, :] + em[:, 0]
    for t in range(1, em.shape[1]):
        x = alpha[:, :, None] + tr.astype(np.float64)[None, :, :] + em[:, t][:, None, :]
        m = x.max(axis=1, keepdims=True)
        nxt = np.log(np.exp(x - m).sum(axis=1)) + m[:, 0, :]
        alpha = np.where(maskb[:, t][:, None], nxt, alpha)
    x = alpha + en.astype(np.float64)[None, :]
    m = x.max(axis=1, keepdims=True)
    denom = np.log(np.exp(x - m).sum(axis=1)) + m[:, 0]
    return np.asarray((denom - num).sum(), dtype=np.float32)


def kernel(**inputs):
    from concourse import bass_utils

    hs = np.asarray(inputs["hidden_states"], dtype=np.float32)
    mask = np.asarray(inputs["attention_mask"])
    labels = np.asarray(inputs["labels"]).astype(np.int64)
    W = np.asarray(inputs["W"], dtype=np.float32)
    bb = np.asarray(inputs["b"], dtype=np.float32)
    st = np.asarray(inputs["start_trans"], dtype=np.float32)
    en = np.asarray(inputs["end_trans"], dtype=np.float32)
    tr = np.asarray(inputs["trans"], dtype=np.float32)

    if not np.all(mask == 1):
        return _numpy_reference(hs, mask, labels, W, bb, st, en, tr)

    expT, g4, ra, rb, iota, id128 = _host_consts(tr)
    xT = np.ascontiguousarray(hs.transpose(0, 2, 1))            # [B, H, S]
    labf = labels.astype(np.float32)
    lab9 = np.ascontiguousarray(
        np.broadcast_to(labf[:, None, :], (B, L, S))
    )                                                            # [B, 9, S]

    nc = _get_nc()
    in_maps = []
    for k in range(NCORES):
        sl = slice(k * BPC, (k + 1) * BPC)
        in_maps.append(
            {
                "xT": xT[sl],
                "Wt": W,
                "lab9": lab9[sl],
                "G4": g4,
                "Ra": ra,
                "Rb": rb,
                "iota9": iota,
                "id128": id128,
            }
        )
    res = bass_utils.run_bass_kernel_spmd(nc, in_maps, list(range(NCORES)))
    _CACHE["last_results"] = res

    # ---- host combine (f64, tiny) ----
    expT64 = np.exp(tr.astype(np.float64))
    e_end = np.exp(en.astype(np.float64))
    e_sb = np.exp((st + bb).astype(np.float64))
    total = 0.0
    for k in range(NCORES):
        r = res.results[k]
        Sf = r["S_out"].astype(np.float64).reshape(BPC, NCHUNK, L, L)
        mv = r["m_out"].astype(np.float64).reshape(BPC, NCHUNK, NNORM)
        E01 = r["e_out"].astype(np.float64)          # [BPC, 9, 2]
        total -= float(r["nt_out"].astype(np.float64).sum())
        for b in range(BPC):
            v = E01[b, :, 0] * e_sb                  # v0 = exp(em_0 + b + start)
            logacc = 0.0
            for c in range(NCHUNK):
                v = v @ Sf[b, c]
                m = v.max()
                v /= m
                logacc += np.log(m)
            v = (v @ expT64) * E01[b, :, 1]          # tail step t = S-1
            denom = np.log(v @ e_end) + logacc + np.log(mv[b]).sum()
            total += denom
        lb = labels[k * BPC : (k + 1) * BPC]
        total -= float(
            st.astype(np.float64)[lb[:, 0]].sum()
            + en.astype(np.float64)[lb[:, -1]].sum()
            + tr.astype(np.float64)[lb[:, :-1], lb[:, 1:]].sum()
            + bb.astype(np.float64)[lb].sum()
        )
    return np.asarray(total, dtype=np.float32)



# revision 7
# speedup vs baseline: 1.4115x; 1.4115x over previous
"""BERT+CRF loss (torchcrf-style, reduction=sum) on 8 Trainium2 NeuronCores.

Strategy (pure data parallel, batch sharded 8 ways, 8 sequences per core):
  emissions^T = W^T @ X^T on TensorE (X pre-transposed + cast to bf16/fp8 on
  host).  Raw emissions^T [9,S] are downloaded (bf16) and the CRF numerator
  (gold-path score) is computed on host.  CRF forward recurrence in exp space:
      v_t = (v_{t-1}^T expT) * E_t,  E_t = exp(em_t)
  Adjacent steps are paired into 9x9 transfer matrices
      B_p[i,j] = sum_k expT[i,k] E_{2p+1}[k] expT[k,j] E_{2p+2}[j]
  computed on TensorE as  outer(E_a, E_b) [81] x G4 [81,81]  (G4 is a host
  constant built from exp(trans)).  Each sequence's 255 pair matrices are
  split into 16 chunks of 16; a chunk-parallel matrix product runs on
  VectorE in bf16 with 128 partitions = 8 batches x 16 chunks, 15 steps
  (state initialized from step 0), periodic max-normalization for range
  safety.  Pair matrices reach the chunk layout via direct SBUF->SBUF DMA
  (no DRAM bounce).  Host combines the 16 chunk matrices per sequence
  (O(B*16*81) f64) and adds the label-indexed numerator terms.
"""

import sys

if "/opt/trn_rl_repo" not in sys.path:
    sys.path.insert(0, "/opt/trn_rl_repo")

import numpy as np

B, S, H, L = 64, 512, 768, 9
NCORES = 8
BPC = B // NCORES          # sequences per core
LL = L * L                 # 81
NPAIR = 256                # pair slots per sequence (255 real + 1 identity)
NCHUNK = 16                # chunks per sequence
SPC = NPAIR // NCHUNK      # pair-steps per chunk = 16
HC = H // 128              # 6 contraction chunks of 128
NORM_STEPS = (5, 11)       # recurrence steps after which we renormalize
NNORM = len(NORM_STEPS)
EM_FP8 = False             # emissions matmul in fp8e4 DoubleRow (W scaled)
WSCALE = 64.0              # fp8 W prescale (undone in exp + host)

_CACHE = {}


def _build_bass():
    import concourse.bass as bass
    import concourse.bacc as bacc
    import concourse.mybir as mybir
    import concourse.tile as tile
    from contextlib import ExitStack

    f32 = mybir.dt.float32
    bf16 = mybir.dt.bfloat16
    em_dt = mybir.dt.float8e4 if EM_FP8 else bf16
    Alu = mybir.AluOpType
    Act = mybir.ActivationFunctionType
    Ax = mybir.AxisListType

    nc = bacc.Bacc()

    # ---- I/O (all host-prearranged, dense layouts) ----
    xT_d = nc.dram_tensor("xT", [BPC, 128, HC * S], em_dt, kind="ExternalInput")
    w_d = nc.dram_tensor("Wt", [128, HC * L], em_dt, kind="ExternalInput")
    g4_d = nc.dram_tensor("G4", [LL, LL], bf16, kind="ExternalInput")
    ra_d = nc.dram_tensor("Ra", [L, LL], bf16, kind="ExternalInput")
    rb_d = nc.dram_tensor("Rb", [L, LL], bf16, kind="ExternalInput")
    id8_d = nc.dram_tensor("Id8", [8, LL], bf16, kind="ExternalInput")

    em_out = nc.dram_tensor("em_out", [BPC, L, S], bf16, kind="ExternalOutput")
    s_out = nc.dram_tensor("S_out", [128, LL], bf16, kind="ExternalOutput")
    m_out = nc.dram_tensor("m_out", [128, NNORM], f32, kind="ExternalOutput")

    with ExitStack() as ctx:
        tc = ctx.enter_context(tile.TileContext(nc))
        const = ctx.enter_context(tc.tile_pool(name="const", bufs=1))
        xpool = ctx.enter_context(tc.tile_pool(name="x", bufs=4))
        epool = ctx.enter_context(tc.tile_pool(name="e", bufs=2))
        spool = ctx.enter_context(tc.tile_pool(name="sm", bufs=3))
        rpool = ctx.enter_context(tc.tile_pool(name="rec", bufs=1))
        dpool = ctx.enter_context(tc.tile_pool(name="dram", bufs=1, space="DRAM"))
        ps_em = ctx.enter_context(tc.tile_pool(name="psem", bufs=3, space="PSUM"))
        ps_rep = ctx.enter_context(tc.tile_pool(name="psrep", bufs=1, space="PSUM"))
        ps_b = ctx.enter_context(tc.tile_pool(name="psb", bufs=2, space="PSUM"))

        # ---- constants into SBUF (already target dtype on host) ----
        w_sb = const.tile([128, HC * L], em_dt)
        nc.gpsimd.dma_start(w_sb[:], w_d[:])
        g4_sb = const.tile([LL, LL], bf16)
        nc.gpsimd.dma_start(g4_sb[:], g4_d[:])
        ra_sb = const.tile([L, LL], bf16)
        nc.gpsimd.dma_start(ra_sb[:], ra_d[:])
        rb_sb = const.tile([L, LL], bf16)
        nc.gpsimd.dma_start(rb_sb[:], rb_d[:])

        # ---- persistent recurrence state ----
        s_tile = rpool.tile([128, LL], bf16)           # chunk-product state
        bc_tile = rpool.tile([128, SPC * LL], bf16)    # pair matrices, chunk layout
        tmp729 = rpool.tile([128, L * L * L], bf16)
        mvals = rpool.tile([128, NNORM], f32)          # applied reciprocal scales

        # DRAM bounce for the pair-layout -> chunk-layout regroup (bf16);
        # row 255 of each sequence = identity (the unwritten filler pair)
        b_all = dpool.tile([BPC, NPAIR, LL], bf16)
        for q in range(BPC):
            nc.scalar.dma_start(b_all[q, NPAIR - 1, :], id8_d[q, :])

        for b in range(BPC):
            # stream X^T for this sequence (two queue-spread DMAs)
            xt = xpool.tile([128, HC * S], em_dt)
            nc.sync.dma_start(xt[:, 0 : 3 * S], xT_d[b, :, 0 : 3 * S])
            nc.sync.dma_start(xt[:, 3 * S : 6 * S], xT_d[b, :, 3 * S : 6 * S])

            # emissions^T [9, S] in PSUM (no bias; handled on host)
            em_ps = ps_em.tile([L, S], f32)
            if EM_FP8:
                for c in range(HC // 2):
                    nc.tensor.matmul(
                        em_ps[:],
                        w_sb[:, 2 * c * L : (2 * c + 2) * L].rearrange(
                            "k (t l) -> k t l", t=2
                        ),
                        xt[:, 2 * c * S : (2 * c + 2) * S].rearrange(
                            "k (t s) -> k t s", t=2
                        ),
                        start=(c == 0),
                        stop=(c == HC // 2 - 1),
                        perf_mode=mybir.MatmulPerfMode.DoubleRow,
                    )
            else:
                for c in range(HC):
                    nc.tensor.matmul(
                        em_ps[:],
                        w_sb[:, c * L : (c + 1) * L],
                        xt[:, c * S : (c + 1) * S],
                        start=(c == 0),
                        stop=(c == HC - 1),
                    )

            # raw emissions download (host computes numerator + v0 + tail)
            em_bf = spool.tile([L, S], bf16)
            nc.vector.tensor_copy(em_bf[:], em_ps[:])
            nc.gpsimd.dma_start(em_out[b], em_bf[:])

            # E = exp(em) in bf16, with one extra zero column at index S
            e_sb = epool.tile([L, S + 1], bf16)
            nc.vector.memset(e_sb[:, S : S + 1], 0.0)
            nc.scalar.activation(
                e_sb[:, 0:S], em_ps[:], Act.Exp, scale=1.0 / WSCALE if EM_FP8 else 1.0
            )

            # replicate E_odd / E_even into [81, 256] via TensorE
            ap0 = e_sb[:].ap[0]
            ea_ap = bass.AP(e_sb.tensor, e_sb[:].offset + 1, [[ap0[0], L], [2, NPAIR]])
            eb_ap = bass.AP(e_sb.tensor, e_sb[:].offset + 2, [[ap0[0], L], [2, NPAIR]])
            earep = ps_rep.tile([LL, NPAIR], f32)
            nc.tensor.matmul(earep[:], ra_sb[:], ea_ap, start=True, stop=True)
            ebrep = ps_rep.tile([LL, NPAIR], f32)
            nc.tensor.matmul(ebrep[:], rb_sb[:], eb_ap, start=True, stop=True)
            eacp = spool.tile([LL, NPAIR], bf16)
            nc.scalar.copy(eacp[:], earep[:])
            ebcp = spool.tile([LL, NPAIR], bf16)
            nc.scalar.copy(ebcp[:], ebrep[:])
            outer = spool.tile([LL, NPAIR], bf16)
            nc.vector.tensor_mul(outer[:], eacp[:], ebcp[:])

            # pair matrices B_p = outer^T @ G4, two halves of 128 pairs,
            # then straight into chunk layout via SBUF->SBUF DMA
            for h in range(2):
                bp = ps_b.tile([128, LL], f32)
                nc.tensor.matmul(
                    bp[:], outer[:, h * 128 : (h + 1) * 128], g4_sb[:],
                    start=True, stop=True,
                )
                bsb = spool.tile([128, LL], bf16)
                nc.vector.tensor_copy(bsb[:], bp[:])
                rows = 128 if h == 0 else 127   # skip pair 255 (stays identity)
                nc.sync.dma_start(
                    b_all[b, h * 128 : h * 128 + rows, :], bsb[0:rows, :]
                )
            # chunk-layout rows for this sequence: partition 16*b+c
            nc.scalar.dma_start(
                bc_tile[16 * b : 16 * (b + 1), :],
                b_all[b].rearrange("(c s) j -> c (s j)", c=NCHUNK),
            )

        # ---- chunk-parallel matrix recurrence: S <- S @ B_s (bf16) ----
        nc.vector.tensor_copy(s_tile[:], bc_tile[:, 0:LL])
        ncol = 0
        for s in range(1, SPC):
            bs = bc_tile[:, s * LL : (s + 1) * LL]
            in0 = (
                s_tile[:].rearrange("p (i k) -> p i k", i=L)
                .unsqueeze(2).broadcast_to([128, L, L, L])
            )
            # bc stores B^T (column-major B): inner k is contiguous
            in1 = (
                bs.rearrange("p (j k) -> p j k", j=L)
                .unsqueeze(1).broadcast_to([128, L, L, L])
            )
            t3 = tmp729[:].rearrange("p (i j k) -> p i j k", i=L, j=L)
            nc.vector.tensor_tensor(out=t3, in0=in0, in1=in1, op=Alu.mult)
            with nc.allow_low_precision(reason="9-term sums; host chains in f64"):
                nc.vector.tensor_reduce(
                    out=s_tile[:], in_=t3, axis=Ax.X, op=Alu.add
                )
            if s in NORM_STEPS:
                mc = spool.tile([128, 1], f32)
                nc.vector.reduce_max(mc[:], s_tile[:], axis=Ax.X)
                rec = mvals[:, ncol : ncol + 1]
                ncol += 1
                nc.vector.reciprocal(rec, mc[:])
                nc.vector.tensor_scalar_mul(s_tile[:], s_tile[:], rec)

        nc.sync.dma_start(s_out[:], s_tile[:])
        nc.sync.dma_start(m_out[:], mvals[:])

    if not nc.is_finalized():
        nc.finalize()
    return nc


def _get_nc():
    if "nc" not in _CACHE:
        _CACHE["nc"] = _build_bass()
    return _CACHE["nc"]


def _host_consts(trans):
    import ml_dtypes

    bf = ml_dtypes.bfloat16
    expT = np.exp(trans.astype(np.float64)).astype(np.float32)  # [9,9]
    k_idx = np.arange(LL) // L   # row index of the 81-flat (k, jb)
    jb_idx = np.arange(LL) % L
    i_idx = np.arange(LL) // L   # col index of the 81-flat (i, j)
    j_idx = np.arange(LL) % L
    # G4[(k,jb),(i,j)] = expT[i,k] * expT[k,j] * (j == jb)
    g4 = (
        expT[np.ix_(i_idx, k_idx)].T
        * expT[np.ix_(k_idx, j_idx)]
        * (j_idx[None, :] == jb_idx[:, None])
    ).astype(np.float32)
    # store B transposed (column-major) so the recurrence reads contiguously
    g4 = np.ascontiguousarray(
        g4.reshape(LL, L, L).swapaxes(1, 2).reshape(LL, LL)
    ).astype(bf)
    ra = (k_idx[None, :] == np.arange(L)[:, None]).astype(bf)   # [9,81]
    rb = (jb_idx[None, :] == np.arange(L)[:, None]).astype(bf)  # [9,81]
    id8 = np.tile(np.eye(L, dtype=np.float32).reshape(1, LL), (8, 1)).astype(bf)
    return expT, g4, ra, rb, id8


def _numpy_reference(hs, mask, labels, W, bb, st, en, tr):
    # general fallback (only used when attention_mask is not all ones)
    em = hs.astype(np.float64) @ W.astype(np.float64) + bb.astype(np.float64)
    maskb = mask.astype(bool)
    maskf = mask.astype(np.float64)
    em_tag = np.take_along_axis(em, labels[..., None], axis=-1)[..., 0]
    num = st.astype(np.float64)[labels[:, 0]] + em_tag[:, 0]
    trs = tr.astype(np.float64)[labels[:, :-1], labels[:, 1:]]
    num = num + np.sum((trs + em_tag[:, 1:]) * maskf[:, 1:], axis=1)
    last = mask.sum(axis=1).astype(np.int64) - 1
    num = num + en.astype(np.float64)[labels[np.arange(len(labels)), last]]
    alpha = st.astype(np.float64)[None, :] + em[:, 0]
    for t in range(1, em.shape[1]):
        x = alpha[:, :, None] + tr.astype(np.float64)[None, :, :] + em[:, t][:, None, :]
        m = x.max(axis=1, keepdims=True)
        nxt = np.log(np.exp(x - m).sum(axis=1)) + m[:, 0, :]
        alpha = np.where(maskb[:, t][:, None], nxt, alpha)
    x = alpha + en.astype(np.float64)[None, :]
    m = x.max(axis=1, keepdims=True)
    denom = np.log(np.exp(x - m).sum(axis=1)) + m[:, 0]
    return np.asarray((denom - num).sum(), dtype=np.float32)


def kernel(**inputs):
    import ml_dtypes
    from concourse import bass_utils

    hs = np.asarray(inputs["hidden_states"], dtype=np.float32)
    mask = np.asarray(inputs["attention_mask"])
    labels = np.asarray(inputs["labels"]).astype(np.int64)
    W = np.asarray(inputs["W"], dtype=np.float32)
    bb = np.asarray(inputs["b"], dtype=np.float32)
    st = np.asarray(inputs["start_trans"], dtype=np.float32)
    en = np.asarray(inputs["end_trans"], dtype=np.float32)
    tr = np.asarray(inputs["trans"], dtype=np.float32)

    if not np.all(mask == 1):
        return _numpy_reference(hs, mask, labels, W, bb, st, en, tr)

    em_np = ml_dtypes.float8_e4m3 if EM_FP8 else ml_dtypes.bfloat16
    expT, g4, ra, rb, id8 = _host_consts(tr)

    # X^T in matmul layout: [B, 128, HC*S], partition k holds H rows c*128+k
    if EM_FP8:
        xc = hs.astype(em_np)
    else:
        xc = hs.astype(em_np)
    xT = np.ascontiguousarray(
        xc.reshape(B, S, HC, 128).transpose(0, 3, 2, 1)
    ).reshape(B, 128, HC * S)
    ws = (W * WSCALE) if EM_FP8 else W
    wT = np.ascontiguousarray(
        ws.reshape(HC, 128, L).transpose(1, 0, 2)
    ).astype(em_np).reshape(128, HC * L)

    nc = _get_nc()
    in_maps = []
    for k in range(NCORES):
        sl = slice(k * BPC, (k + 1) * BPC)
        in_maps.append(
            {
                "xT": xT[sl],
                "Wt": wT,
                "G4": g4,
                "Ra": ra,
                "Rb": rb,
                "Id8": id8,
            }
        )
    res = bass_utils.run_bass_kernel_spmd(nc, in_maps, list(range(NCORES)))
    _CACHE["last_results"] = res

    # ---- host combine (f64, tiny) ----
    expT64 = np.exp(tr.astype(np.float64))
    e_end = np.exp(en.astype(np.float64))
    st64 = st.astype(np.float64)
    bb64 = bb.astype(np.float64)
    en64 = en.astype(np.float64)
    tr64 = tr.astype(np.float64)
    total = 0.0
    for k in range(NCORES):
        r = res.results[k]
        em = r["em_out"].astype(np.float64)          # [BPC, 9, S]
        if EM_FP8:
            em = em / WSCALE
        Sf = r["S_out"].astype(np.float64).reshape(BPC, NCHUNK, L, L)
        mv = r["m_out"].astype(np.float64).reshape(BPC, NCHUNK, NNORM)
        for b in range(BPC):
            v = np.exp(em[b, :, 0] + st64 + bb64)    # v0
            logacc = -np.log(mv[b]).sum()            # undo applied scales
            for c in range(NCHUNK):
                v = v @ Sf[b, c]
                m = v.max()
                v /= m
                logacc += np.log(m)
            v = (v @ expT64) * np.exp(em[b, :, S - 1] + bb64)  # tail t = S-1
            total += np.log(v @ e_end) + logacc
        # numerator for this core's sequences (gold path score)
        lb = labels[k * BPC : (k + 1) * BPC]
        em_tag = np.take_along_axis(em, lb[:, None, :], axis=1)[:, 0, :]  # [BPC,S]
        total -= float(
            em_tag.sum()
            + st64[lb[:, 0]].sum()
            + en64[lb[:, -1]].sum()
            + tr64[lb[:, :-1], lb[:, 1:]].sum()
            + bb64[lb].sum()
        )
    return np.asarray(total, dtype=np.float32)


# revision 12
# speedup vs baseline: 1.5941x; 1.1293x over previous
"""BERT+CRF loss (torchcrf-style, reduction=sum) on 8 Trainium2 NeuronCores.

Strategy (pure data parallel, batch sharded 8 ways, 8 sequences per core):
  emissions^T = W^T @ X^T on TensorE (X pre-transposed + cast to bf16/fp8 on
  host).  Raw emissions^T [9,S] are downloaded (bf16) and the CRF numerator
  (gold-path score) is computed on host.  CRF forward recurrence in exp space:
      v_t = (v_{t-1}^T expT) * E_t,  E_t = exp(em_t)
  Adjacent steps are paired into 9x9 transfer matrices
      B_p[i,j] = sum_k expT[i,k] E_{2p+1}[k] expT[k,j] E_{2p+2}[j]
  computed on TensorE as  outer(E_a, E_b) [81] x G4 [81,81]  (G4 is a host
  constant built from exp(trans)).  Each sequence's 255 pair matrices are
  split into 16 chunks of 16; a chunk-parallel matrix product runs on
  VectorE in bf16 with 128 partitions = 8 batches x 16 chunks, 15 steps
  (state initialized from step 0), periodic max-normalization for range
  safety.  Pair matrices reach the chunk layout via direct SBUF->SBUF DMA
  (no DRAM bounce).  Host combines the 16 chunk matrices per sequence
  (O(B*16*81) f64) and adds the label-indexed numerator terms.
"""

import sys

if "/opt/trn_rl_repo" not in sys.path:
    sys.path.insert(0, "/opt/trn_rl_repo")

import numpy as np

B, S, H, L = 64, 512, 768, 9
NCORES = 8
BPC = B // NCORES          # sequences per core
LL = L * L                 # 81
NPAIR = 256                # pair slots per sequence (255 real + 1 identity)
NCHUNK = 16                # chunks per sequence
SPC = NPAIR // NCHUNK      # pair-steps per chunk = 16
HC = H // 128              # 6 contraction chunks of 128
NORM_STEPS = (5, 11)       # recurrence steps after which we renormalize
NNORM = len(NORM_STEPS)
EM_FP8 = True              # emissions matmul in fp8e4 DoubleRow (W scaled)
WSCALE = 64.0              # fp8 W prescale (undone in exp + host)

_CACHE = {}


def _build_bass():
    import concourse.bass as bass
    import concourse.bacc as bacc
    import concourse.mybir as mybir
    import concourse.tile as tile
    from contextlib import ExitStack

    f32 = mybir.dt.float32
    bf16 = mybir.dt.bfloat16
    em_dt = mybir.dt.float8e4 if EM_FP8 else bf16
    Alu = mybir.AluOpType
    Act = mybir.ActivationFunctionType
    Ax = mybir.AxisListType

    nc = bacc.Bacc()

    # ---- I/O (all host-prearranged, dense layouts) ----
    # weight rows padded to 16 elems/chunk: DoubleRow needs dual-row step%16==0
    WP = 16 if EM_FP8 else L
    xT_d = nc.dram_tensor("xT", [BPC, 128, HC * S], em_dt, kind="ExternalInput")
    w_d = nc.dram_tensor("Wt", [128, HC * WP], em_dt, kind="ExternalInput")
    g4_d = nc.dram_tensor("G4", [LL, LL], bf16, kind="ExternalInput")
    ra_d = nc.dram_tensor("Ra", [L, LL], bf16, kind="ExternalInput")
    rb_d = nc.dram_tensor("Rb", [L, LL], bf16, kind="ExternalInput")
    id8_d = nc.dram_tensor("Id8", [8, LL], bf16, kind="ExternalInput")

    em_out = nc.dram_tensor("em_out", [BPC, L, S], bf16, kind="ExternalOutput")
    s_out = nc.dram_tensor("S_out", [128, LL], bf16, kind="ExternalOutput")
    m_out = nc.dram_tensor("m_out", [128, NNORM], f32, kind="ExternalOutput")

    with ExitStack() as ctx:
        tc = ctx.enter_context(tile.TileContext(nc))
        const = ctx.enter_context(tc.tile_pool(name="const", bufs=1))
        xpool = ctx.enter_context(tc.tile_pool(name="x", bufs=4))
        epool = ctx.enter_context(tc.tile_pool(name="e", bufs=2))
        spool = ctx.enter_context(tc.tile_pool(name="sm", bufs=3))
        rpool = ctx.enter_context(tc.tile_pool(name="rec", bufs=1))
        dpool = ctx.enter_context(tc.tile_pool(name="dram", bufs=1, space="DRAM"))
        ps_em = ctx.enter_context(tc.tile_pool(name="psem", bufs=3, space="PSUM"))
        ps_rep = ctx.enter_context(tc.tile_pool(name="psrep", bufs=1, space="PSUM"))
        ps_b = ctx.enter_context(tc.tile_pool(name="psb", bufs=2, space="PSUM"))

        # ---- constants into SBUF (already target dtype on host) ----
        w_sb = const.tile([128, HC * WP], em_dt)
        nc.gpsimd.dma_start(w_sb[:], w_d[:])
        g4_sb = const.tile([LL, LL], bf16)
        nc.gpsimd.dma_start(g4_sb[:], g4_d[:])
        ra_sb = const.tile([L, LL], bf16)
        nc.gpsimd.dma_start(ra_sb[:], ra_d[:])
        rb_sb = const.tile([L, LL], bf16)
        nc.gpsimd.dma_start(rb_sb[:], rb_d[:])

        # ---- persistent recurrence state ----
        s_tile = rpool.tile([128, LL], bf16)           # chunk-product state
        bc_tile = rpool.tile([128, SPC * LL], bf16)    # pair matrices, chunk layout
        tmp729 = rpool.tile([128, L * L * L], bf16)
        mvals = rpool.tile([128, NNORM], f32)          # applied reciprocal scales

        # DRAM bounce for the pair-layout -> chunk-layout regroup (bf16);
        # row 255 of each sequence = identity (the unwritten filler pair)
        b_all = dpool.tile([BPC, NPAIR, LL], bf16)
        for q in range(BPC):
            nc.scalar.dma_start(b_all[q, NPAIR - 1, :], id8_d[q, :])

        for b in range(BPC):
            # stream X^T for this sequence (two queue-spread DMAs)
            xt = xpool.tile([128, HC * S], em_dt)
            nc.sync.dma_start(xt[:, 0 : 3 * S], xT_d[b, :, 0 : 3 * S])
            nc.sync.dma_start(xt[:, 3 * S : 6 * S], xT_d[b, :, 3 * S : 6 * S])

            # emissions^T [9, S] in PSUM (no bias; handled on host)
            em_ps = ps_em.tile([L, S], f32)
            if EM_FP8:
                for c in range(HC // 2):
                    nc.tensor.matmul(
                        em_ps[:],
                        w_sb[:, 2 * c * WP : (2 * c + 2) * WP].rearrange(
                            "k (t l) -> k t l", t=2
                        )[:, :, 0:L],
                        xt[:, 2 * c * S : (2 * c + 2) * S].rearrange(
                            "k (t s) -> k t s", t=2
                        ),
                        start=(c == 0),
                        stop=(c == HC // 2 - 1),
                        perf_mode=mybir.MatmulPerfMode.DoubleRow,
                    )
            else:
                for c in range(HC):
                    nc.tensor.matmul(
                        em_ps[:],
                        w_sb[:, c * L : (c + 1) * L],
                        xt[:, c * S : (c + 1) * S],
                        start=(c == 0),
                        stop=(c == HC - 1),
                    )

            # raw emissions download (host computes numerator + v0 + tail)
            em_bf = spool.tile([L, S], bf16)
            nc.vector.tensor_copy(em_bf[:], em_ps[:])
            nc.gpsimd.dma_start(em_out[b], em_bf[:])

            # E = exp(em) in bf16, with one extra zero column at index S
            e_sb = epool.tile([L, S + 1], bf16)
            nc.vector.memset(e_sb[:, S : S + 1], 0.0)
            nc.scalar.activation(
                e_sb[:, 0:S], em_ps[:], Act.Exp, scale=1.0 / WSCALE if EM_FP8 else 1.0
            )

            # replicate E_odd / E_even into [81, 256] via TensorE
            ap0 = e_sb[:].ap[0]
            ea_ap = bass.AP(e_sb.tensor, e_sb[:].offset + 1, [[ap0[0], L], [2, NPAIR]])
            eb_ap = bass.AP(e_sb.tensor, e_sb[:].offset + 2, [[ap0[0], L], [2, NPAIR]])
            earep = ps_rep.tile([LL, NPAIR], f32)
            nc.tensor.matmul(earep[:], ra_sb[:], ea_ap, start=True, stop=True)
            ebrep = ps_rep.tile([LL, NPAIR], f32)
            nc.tensor.matmul(ebrep[:], rb_sb[:], eb_ap, start=True, stop=True)
            eacp = spool.tile([LL, NPAIR], bf16)
            nc.scalar.copy(eacp[:], earep[:])
            ebcp = spool.tile([LL, NPAIR], bf16)
            nc.scalar.copy(ebcp[:], ebrep[:])
            outer = spool.tile([LL, NPAIR], bf16)
            nc.vector.tensor_mul(outer[:], eacp[:], ebcp[:])

            # pair matrices B_p = outer^T @ G4, two halves of 128 pairs,
            # then straight into chunk layout via SBUF->SBUF DMA
            for h in range(2):
                bp = ps_b.tile([128, LL], f32)
                nc.tensor.matmul(
                    bp[:], outer[:, h * 128 : (h + 1) * 128], g4_sb[:],
                    start=True, stop=True,
                )
                bsb = spool.tile([128, LL], bf16)
                nc.vector.tensor_copy(bsb[:], bp[:])
                rows = 128 if h == 0 else 127   # skip pair 255 (stays identity)
                nc.sync.dma_start(
                    b_all[b, h * 128 : h * 128 + rows, :], bsb[0:rows, :]
                )
            # chunk-layout rows for this sequence: partition 16*b+c
            nc.scalar.dma_start(
                bc_tile[16 * b : 16 * (b + 1), :],
                b_all[b].rearrange("(c s) j -> c (s j)", c=NCHUNK),
            )

        # ---- chunk-parallel matrix recurrence: S <- S @ B_s (bf16) ----
        nc.vector.tensor_copy(s_tile[:], bc_tile[:, 0:LL])
        ncol = 0
        for s in range(1, SPC):
            bs = bc_tile[:, s * LL : (s + 1) * LL]
            in0 = (
                s_tile[:].rearrange("p (i k) -> p i k", i=L)
                .unsqueeze(2).broadcast_to([128, L, L, L])
            )
            # bc stores B^T (column-major B): inner k is contiguous
            in1 = (
                bs.rearrange("p (j k) -> p j k", j=L)
                .unsqueeze(1).broadcast_to([128, L, L, L])
            )
            t3 = tmp729[:].rearrange("p (i j k) -> p i j k", i=L, j=L)
            nc.vector.tensor_tensor(out=t3, in0=in0, in1=in1, op=Alu.mult)
            with nc.allow_low_precision(reason="9-term sums; host chains in f64"):
                nc.vector.tensor_reduce(
                    out=s_tile[:], in_=t3, axis=Ax.X, op=Alu.add
                )
            if s in NORM_STEPS:
                mc = spool.tile([128, 1], f32)
                nc.vector.reduce_max(mc[:], s_tile[:], axis=Ax.X)
                rec = mvals[:, ncol : ncol + 1]
                ncol += 1
                nc.vector.reciprocal(rec, mc[:])
                nc.vector.tensor_scalar_mul(s_tile[:], s_tile[:], rec)

        nc.sync.dma_start(s_out[:], s_tile[:])
        nc.sync.dma_start(m_out[:], mvals[:])

    if not nc.is_finalized():
        nc.finalize()
    return nc


def _get_nc():
    if "nc" not in _CACHE:
        _CACHE["nc"] = _build_bass()
    return _CACHE["nc"]


def _host_consts(trans):
    import ml_dtypes

    bf = ml_dtypes.bfloat16
    expT = np.exp(trans.astype(np.float64)).astype(np.float32)  # [9,9]
    k_idx = np.arange(LL) // L   # row index of the 81-flat (k, jb)
    jb_idx = np.arange(LL) % L
    i_idx = np.arange(LL) // L   # col index of the 81-flat (i, j)
    j_idx = np.arange(LL) % L
    # G4[(k,jb),(i,j)] = expT[i,k] * expT[k,j] * (j == jb)
    g4 = (
        expT[np.ix_(i_idx, k_idx)].T
        * expT[np.ix_(k_idx, j_idx)]
        * (j_idx[None, :] == jb_idx[:, None])
    ).astype(np.float32)
    # store B transposed (column-major) so the recurrence reads contiguously
    g4 = np.ascontiguousarray(
        g4.reshape(LL, L, L).swapaxes(1, 2).reshape(LL, LL)
    ).astype(bf)
    ra = (k_idx[None, :] == np.arange(L)[:, None]).astype(bf)   # [9,81]
    rb = (jb_idx[None, :] == np.arange(L)[:, None]).astype(bf)  # [9,81]
    id8 = np.tile(np.eye(L, dtype=np.float32).reshape(1, LL), (8, 1)).astype(bf)
    return expT, g4, ra, rb, id8


def _numpy_reference(hs, mask, labels, W, bb, st, en, tr):
    # general fallback (only used when attention_mask is not all ones)
    em = hs.astype(np.float64) @ W.astype(np.float64) + bb.astype(np.float64)
    maskb = mask.astype(bool)
    maskf = mask.astype(np.float64)
    em_tag = np.take_along_axis(em, labels[..., None], axis=-1)[..., 0]
    num = st.astype(np.float64)[labels[:, 0]] + em_tag[:, 0]
    trs = tr.astype(np.float64)[labels[:, :-1], labels[:, 1:]]
    num = num + np.sum((trs + em_tag[:, 1:]) * maskf[:, 1:], axis=1)
    last = mask.sum(axis=1).astype(np.int64) - 1
    num = num + en.astype(np.float64)[labels[np.arange(len(labels)), last]]
    alpha = st.astype(np.float64)[None, :] + em[:, 0]
    for t in range(1, em.shape[1]):
        x = alpha[:, :, None] + tr.astype(np.float64)[None, :, :] + em[:, t][:, None, :]
        m = x.max(axis=1, keepdims=True)
        nxt = np.log(np.exp(x - m).sum(axis=1)) + m[:, 0, :]
        alpha = np.where(maskb[:, t][:, None], nxt, alpha)
    x = alpha + en.astype(np.float64)[None, :]
    m = x.max(axis=1, keepdims=True)
    denom = np.log(np.exp(x - m).sum(axis=1)) + m[:, 0]
    return np.asarray((denom - num).sum(), dtype=np.float32)


def kernel(**inputs):
    import ml_dtypes
    from concourse import bass_utils

    hs = np.asarray(inputs["hidden_states"], dtype=np.float32)
    mask = np.asarray(inputs["attention_mask"])
    labels = np.asarray(inputs["labels"]).astype(np.int64)
    W = np.asarray(inputs["W"], dtype=np.float32)
    bb = np.asarray(inputs["b"], dtype=np.float32)
    st = np.asarray(inputs["start_trans"], dtype=np.float32)
    en = np.asarray(inputs["end_trans"], dtype=np.float32)
    tr = np.asarray(inputs["trans"], dtype=np.float32)

    if not np.all(mask == 1):
        return _numpy_reference(hs, mask, labels, W, bb, st, en, tr)

    em_np = ml_dtypes.float8_e4m3 if EM_FP8 else ml_dtypes.bfloat16
    expT, g4, ra, rb, id8 = _host_consts(tr)

    # X^T in matmul layout: [B, 128, HC*S], partition k holds H rows c*128+k
    if EM_FP8:
        xc = hs.astype(em_np)
    else:
        xc = hs.astype(em_np)
    xT = np.ascontiguousarray(
        xc.reshape(B, S, HC, 128).transpose(0, 3, 2, 1)
    ).reshape(B, 128, HC * S)
    ws = (W * WSCALE) if EM_FP8 else W
    wT = np.ascontiguousarray(
        ws.reshape(HC, 128, L).transpose(1, 0, 2)
    ).astype(em_np)                                   # [128, HC, L]
    if EM_FP8:
        wp = np.zeros((128, HC, 16), dtype=em_np)
        wp[:, :, :L] = wT
        wT = wp
    wT = wT.reshape(128, -1)

    nc = _get_nc()
    in_maps = []
    for k in range(NCORES):
        sl = slice(k * BPC, (k + 1) * BPC)
        in_maps.append(
            {
                "xT": xT[sl],
                "Wt": wT,
                "G4": g4,
                "Ra": ra,
                "Rb": rb,
                "Id8": id8,
            }
        )
    res = bass_utils.run_bass_kernel_spmd(nc, in_maps, list(range(NCORES)))
    _CACHE["last_results"] = res

    # ---- host combine (f64, tiny) ----
    expT64 = np.exp(tr.astype(np.float64))
    e_end = np.exp(en.astype(np.float64))
    st64 = st.astype(np.float64)
    bb64 = bb.astype(np.float64)
    en64 = en.astype(np.float64)
    tr64 = tr.astype(np.float64)
    total = 0.0
    for k in range(NCORES):
        r = res.results[k]
        em = r["em_out"].astype(np.float64)          # [BPC, 9, S]
        if EM_FP8:
            em = em / WSCALE
        Sf = r["S_out"].astype(np.float64).reshape(BPC, NCHUNK, L, L)
        mv = r["m_out"].astype(np.float64).reshape(BPC, NCHUNK, NNORM)
        for b in range(BPC):
            v = np.exp(em[b, :, 0] + st64 + bb64)    # v0
            logacc = -np.log(mv[b]).sum()            # undo applied scales
            for c in range(NCHUNK):
                v = v @ Sf[b, c]
                m = v.max()
                v /= m
                logacc += np.log(m)
            v = (v @ expT64) * np.exp(em[b, :, S - 1] + bb64)  # tail t = S-1
            total += np.log(v @ e_end) + logacc
        # numerator for this core's sequences (gold path score)
        lb = labels[k * BPC : (k + 1) * BPC]
        em_tag = np.take_along_axis(em, lb[:, None, :], axis=1)[:, 0, :]  # [BPC,S]
        total -= float(
            em_tag.sum()
            + st64[lb[:, 0]].sum()
            + en64[lb[:, -1]].sum()
            + tr64[lb[:, :-1], lb[:, 1:]].sum()
            + bb64[lb].sum()
        )
    return np.asarray(total, dtype=np.float32)


# revision 17
# speedup vs baseline: 1.6613x; 1.0421x over previous
"""BERT+CRF loss (torchcrf-style, reduction=sum) on 8 Trainium2 NeuronCores.

Strategy (pure data parallel, batch sharded 8 ways, 8 sequences per core):
  emissions^T = W^T @ X^T on TensorE (X pre-transposed + cast to bf16/fp8 on
  host).  Raw emissions^T [9,S] are downloaded (bf16) and the CRF numerator
  (gold-path score) is computed on host.  CRF forward recurrence in exp space:
      v_t = (v_{t-1}^T expT) * E_t,  E_t = exp(em_t)
  Adjacent steps are paired into 9x9 transfer matrices
      B_p[i,j] = sum_k expT[i,k] E_{2p+1}[k] expT[k,j] E_{2p+2}[j]
  computed on TensorE as  outer(E_a, E_b) [81] x G4 [81,81]  (G4 is a host
  constant built from exp(trans)).  Each sequence's 255 pair matrices are
  split into 16 chunks of 16; a chunk-parallel matrix product runs on
  VectorE in bf16 with 128 partitions = 8 batches x 16 chunks, 15 steps
  (state initialized from step 0), periodic max-normalization for range
  safety.  Pair matrices reach the chunk layout via direct SBUF->SBUF DMA
  (no DRAM bounce).  Host combines the 16 chunk matrices per sequence
  (O(B*16*81) f64) and adds the label-indexed numerator terms.
"""

import sys

if "/opt/trn_rl_repo" not in sys.path:
    sys.path.insert(0, "/opt/trn_rl_repo")

import numpy as np

B, S, H, L = 64, 512, 768, 9
NCORES = 8
BPC = B // NCORES          # sequences per core
LL = L * L                 # 81
NPAIR = 256                # pair slots per sequence (255 real + 1 identity)
NCHUNK = 16                # chunks per sequence
SPC = NPAIR // NCHUNK      # pair-steps per chunk = 16
HC = H // 128              # 6 contraction chunks of 128
NORM_STEPS = (5, 11)       # recurrence steps after which we renormalize
NNORM = len(NORM_STEPS)
EM_FP8 = True              # emissions matmul in fp8e4 DoubleRow (W scaled)
WSCALE = 64.0              # fp8 W prescale (undone in exp + host)

_CACHE = {}


def _build_bass():
    import concourse.bass as bass
    import concourse.bacc as bacc
    import concourse.mybir as mybir
    import concourse.tile as tile
    from contextlib import ExitStack

    f32 = mybir.dt.float32
    bf16 = mybir.dt.bfloat16
    em_dt = mybir.dt.float8e4 if EM_FP8 else bf16
    Alu = mybir.AluOpType
    Act = mybir.ActivationFunctionType
    Ax = mybir.AxisListType

    nc = bacc.Bacc()

    # ---- I/O (all host-prearranged, dense layouts) ----
    # weight rows padded to 16 elems/chunk: DoubleRow needs dual-row step%16==0
    WP = 16 if EM_FP8 else L
    xT_d = nc.dram_tensor("xT", [BPC, 128, HC * S], em_dt, kind="ExternalInput")
    w_d = nc.dram_tensor("Wt", [128, HC * WP], em_dt, kind="ExternalInput")
    g4_d = nc.dram_tensor("G4", [LL, LL], bf16, kind="ExternalInput")
    ra_d = nc.dram_tensor("Ra", [L, LL], bf16, kind="ExternalInput")
    rb_d = nc.dram_tensor("Rb", [L, LL], bf16, kind="ExternalInput")
    id8_d = nc.dram_tensor("Id8", [8, LL], bf16, kind="ExternalInput")

    em_out = nc.dram_tensor("em_out", [BPC, L, S], bf16, kind="ExternalOutput")
    s_out = nc.dram_tensor("S_out", [128, LL], bf16, kind="ExternalOutput")
    m_out = nc.dram_tensor("m_out", [128, NNORM], f32, kind="ExternalOutput")

    with ExitStack() as ctx:
        tc = ctx.enter_context(tile.TileContext(nc))
        const = ctx.enter_context(tc.tile_pool(name="const", bufs=1))
        xpool = ctx.enter_context(tc.tile_pool(name="x", bufs=4))
        epool = ctx.enter_context(tc.tile_pool(name="e", bufs=2))
        spool = ctx.enter_context(tc.tile_pool(name="sm", bufs=3))
        rpool = ctx.enter_context(tc.tile_pool(name="rec", bufs=1))
        dpool = ctx.enter_context(tc.tile_pool(name="dram", bufs=1, space="DRAM"))
        ps_em = ctx.enter_context(tc.tile_pool(name="psem", bufs=3, space="PSUM"))
        ps_rep = ctx.enter_context(tc.tile_pool(name="psrep", bufs=1, space="PSUM"))
        ps_b = ctx.enter_context(tc.tile_pool(name="psb", bufs=2, space="PSUM"))

        # ---- constants into SBUF (already target dtype on host) ----
        w_sb = const.tile([128, HC * WP], em_dt)
        nc.gpsimd.dma_start(w_sb[:], w_d[:])
        g4_sb = const.tile([LL, LL], bf16)
        nc.gpsimd.dma_start(g4_sb[:], g4_d[:])
        ra_sb = const.tile([L, LL], bf16)
        nc.gpsimd.dma_start(ra_sb[:], ra_d[:])
        rb_sb = const.tile([L, LL], bf16)
        nc.gpsimd.dma_start(rb_sb[:], rb_d[:])

        # ---- persistent recurrence state ----
        s_tile = rpool.tile([128, LL], bf16)           # chunk-product state
        bc_tile = rpool.tile([128, SPC * LL], bf16)    # pair matrices, chunk layout
        tmp729 = rpool.tile([128, L * L * L], bf16)
        mvals = rpool.tile([128, NNORM], f32)          # applied reciprocal scales

        # DRAM bounce for the pair-layout -> chunk-layout regroup (bf16);
        # row 255 of each sequence = identity (the unwritten filler pair)
        b_all = dpool.tile([BPC, NPAIR, LL], bf16)
        for q in range(BPC):
            nc.scalar.dma_start(b_all[q, NPAIR - 1, :], id8_d[q, :])

        SP = S + 3  # e_sb column pad (pair col index reaches S; keep 4B align)

        def emissions(b, e2, q):
            """X DMA + emissions matmul + em download + exp for sequence b.
            Writes exp(em) into half q of the shared pair tile e2."""
            xt = xpool.tile([128, HC * S], em_dt)
            if b == 0:
                # split so the first matmul can start after half the DMA
                nc.sync.dma_start(xt[:, 0 : 2 * S], xT_d[b, :, 0 : 2 * S])
                nc.sync.dma_start(xt[:, 2 * S : 6 * S], xT_d[b, :, 2 * S : 6 * S])
            else:
                nc.sync.dma_start(xt[:], xT_d[b])

            em_ps = ps_em.tile([L, S], f32)
            if EM_FP8:
                for c in range(HC // 2):
                    nc.tensor.matmul(
                        em_ps[:],
                        w_sb[:, 2 * c * WP : (2 * c + 2) * WP].rearrange(
                            "k (t l) -> k t l", t=2
                        )[:, :, 0:L],
                        xt[:, 2 * c * S : (2 * c + 2) * S].rearrange(
                            "k (t s) -> k t s", t=2
                        ),
                        start=(c == 0),
                        stop=(c == HC // 2 - 1),
                        perf_mode=mybir.MatmulPerfMode.DoubleRow,
                    )
            else:
                for c in range(HC):
                    nc.tensor.matmul(
                        em_ps[:],
                        w_sb[:, c * L : (c + 1) * L],
                        xt[:, c * S : (c + 1) * S],
                        start=(c == 0),
                        stop=(c == HC - 1),
                    )

            # raw emissions download (host computes numerator + v0 + tail)
            em_bf = spool.tile([L, S], bf16)
            nc.vector.tensor_copy(em_bf[:], em_ps[:])
            nc.gpsimd.dma_start(em_out[b], em_bf[:])

            # E = exp(em) in bf16, with a zero column at index S
            nc.vector.memset(e2[:, q, S:SP], 0.0)
            nc.scalar.activation(
                e2[:, q, 0:S], em_ps[:], Act.Exp,
                scale=1.0 / WSCALE if EM_FP8 else 1.0,
            )

        def pair_block(b, e2):
            """Pair matrices for sequences b, b+1 (one batched replication)."""
            # both sequences' E columns in one moving operand [9, 2, 256]
            pstride = e2[:].ap[0][0]
            off = e2[:].offset
            ea_ap = bass.AP(
                e2.tensor, off + 1, [[pstride, L], [SP, 2], [2, NPAIR]]
            )
            eb_ap = bass.AP(
                e2.tensor, off + 2, [[pstride, L], [SP, 2], [2, NPAIR]]
            )
            earep = ps_rep.tile([LL, 2 * NPAIR], f32)
            nc.tensor.matmul(earep[:], ra_sb[:], ea_ap, start=True, stop=True)
            ebrep = ps_rep.tile([LL, 2 * NPAIR], f32)
            nc.tensor.matmul(ebrep[:], rb_sb[:], eb_ap, start=True, stop=True)
            # one PSUM->SBUF copy, then outer = Ea*Eb (one PSUM read allowed)
            ebcp = spool.tile([LL, 2 * NPAIR], bf16)
            nc.scalar.copy(ebcp[:], ebrep[:])
            outer = spool.tile([LL, 2 * NPAIR], bf16)
            nc.vector.tensor_mul(outer[:], earep[:], ebcp[:])

            for q in range(2):          # sequence within the pair
                bsb = spool.tile([128, 2 * LL], bf16)
                for h in range(2):      # half of the 256 pairs
                    bp = ps_b.tile([128, LL], f32)
                    nc.tensor.matmul(
                        bp[:],
                        outer[:, (2 * q + h) * 128 : (2 * q + h + 1) * 128],
                        g4_sb[:],
                        start=True, stop=True,
                    )
                    nc.vector.tensor_copy(bsb[:, h * LL : (h + 1) * LL], bp[:])
                # bounce: pair-major rows, then chunk-layout read-back
                nc.sync.dma_start(
                    b_all[b + q, 0:128, :], bsb[:, 0:LL]
                )
                nc.gpsimd.dma_start(
                    b_all[b + q, 128:255, :], bsb[0:127, LL : 2 * LL]
                )
                nc.scalar.dma_start(
                    bc_tile[16 * (b + q) : 16 * (b + q + 1), :],
                    b_all[b + q].rearrange("(c s) j -> c (s j)", c=NCHUNK),
                )

        e2 = None
        for b in range(BPC):
            if b % 2 == 0:
                e2 = epool.tile([L, 2, SP], bf16)
            emissions(b, e2, b % 2)
            if b % 2 == 1:
                pair_block(b - 1, e2)

        # ---- chunk-parallel matrix recurrence: S <- S @ B_s (bf16) ----
        nc.vector.tensor_copy(s_tile[:], bc_tile[:, 0:LL])
        ncol = 0
        for s in range(1, SPC):
            bs = bc_tile[:, s * LL : (s + 1) * LL]
            in0 = (
                s_tile[:].rearrange("p (i k) -> p i k", i=L)
                .unsqueeze(2).broadcast_to([128, L, L, L])
            )
            # bc stores B^T (column-major B): inner k is contiguous
            in1 = (
                bs.rearrange("p (j k) -> p j k", j=L)
                .unsqueeze(1).broadcast_to([128, L, L, L])
            )
            t3 = tmp729[:].rearrange("p (i j k) -> p i j k", i=L, j=L)
            nc.vector.tensor_tensor(out=t3, in0=in0, in1=in1, op=Alu.mult)
            with nc.allow_low_precision(reason="9-term sums; host chains in f64"):
                nc.vector.tensor_reduce(
                    out=s_tile[:], in_=t3, axis=Ax.X, op=Alu.add
                )
            if s in NORM_STEPS:
                mc = spool.tile([128, 1], f32)
                nc.vector.reduce_max(mc[:], s_tile[:], axis=Ax.X)
                rec = mvals[:, ncol : ncol + 1]
                ncol += 1
                nc.vector.reciprocal(rec, mc[:])
                nc.vector.tensor_scalar_mul(s_tile[:], s_tile[:], rec)

        nc.sync.dma_start(s_out[:], s_tile[:])
        nc.sync.dma_start(m_out[:], mvals[:])

    if not nc.is_finalized():
        nc.finalize()
    return nc


def _get_nc():
    if "nc" not in _CACHE:
        _CACHE["nc"] = _build_bass()
    return _CACHE["nc"]


def _host_consts(trans):
    import ml_dtypes

    bf = ml_dtypes.bfloat16
    expT = np.exp(trans.astype(np.float64)).astype(np.float32)  # [9,9]
    k_idx = np.arange(LL) // L   # row index of the 81-flat (k, jb)
    jb_idx = np.arange(LL) % L
    i_idx = np.arange(LL) // L   # col index of the 81-flat (i, j)
    j_idx = np.arange(LL) % L
    # G4[(k,jb),(i,j)] = expT[i,k] * expT[k,j] * (j == jb)
    g4 = (
        expT[np.ix_(i_idx, k_idx)].T
        * expT[np.ix_(k_idx, j_idx)]
        * (j_idx[None, :] == jb_idx[:, None])
    ).astype(np.float32)
    # store B transposed (column-major) so the recurrence reads contiguously
    g4 = np.ascontiguousarray(
        g4.reshape(LL, L, L).swapaxes(1, 2).reshape(LL, LL)
    ).astype(bf)
    ra = (k_idx[None, :] == np.arange(L)[:, None]).astype(bf)   # [9,81]
    rb = (jb_idx[None, :] == np.arange(L)[:, None]).astype(bf)  # [9,81]
    id8 = np.tile(np.eye(L, dtype=np.float32).reshape(1, LL), (8, 1)).astype(bf)
    return expT, g4, ra, rb, id8


def _numpy_reference(hs, mask, labels, W, bb, st, en, tr):
    # general fallback (only used when attention_mask is not all ones)
    em = hs.astype(np.float64) @ W.astype(np.float64) + bb.astype(np.float64)
    maskb = mask.astype(bool)
    maskf = mask.astype(np.float64)
    em_tag = np.take_along_axis(em, labels[..., None], axis=-1)[..., 0]
    num = st.astype(np.float64)[labels[:, 0]] + em_tag[:, 0]
    trs = tr.astype(np.float64)[labels[:, :-1], labels[:, 1:]]
    num = num + np.sum((trs + em_tag[:, 1:]) * maskf[:, 1:], axis=1)
    last = mask.sum(axis=1).astype(np.int64) - 1
    num = num + en.astype(np.float64)[labels[np.arange(len(labels)), last]]
    alpha = st.astype(np.float64)[None, :] + em[:, 0]
    for t in range(1, em.shape[1]):
        x = alpha[:, :, None] + tr.astype(np.float64)[None, :, :] + em[:, t][:, None, :]
        m = x.max(axis=1, keepdims=True)
        nxt = np.log(np.exp(x - m).sum(axis=1)) + m[:, 0, :]
        alpha = np.where(maskb[:, t][:, None], nxt, alpha)
    x = alpha + en.astype(np.float64)[None, :]
    m = x.max(axis=1, keepdims=True)
    denom = np.log(np.exp(x - m).sum(axis=1)) + m[:, 0]
    return np.asarray((denom - num).sum(), dtype=np.float32)


def kernel(**inputs):
    import ml_dtypes
    from concourse import bass_utils

    hs = np.asarray(inputs["hidden_states"], dtype=np.float32)
    mask = np.asarray(inputs["attention_mask"])
    labels = np.asarray(inputs["labels"]).astype(np.int64)
    W = np.asarray(inputs["W"], dtype=np.float32)
    bb = np.asarray(inputs["b"], dtype=np.float32)
    st = np.asarray(inputs["start_trans"], dtype=np.float32)
    en = np.asarray(inputs["end_trans"], dtype=np.float32)
    tr = np.asarray(inputs["trans"], dtype=np.float32)

    if not np.all(mask == 1):
        return _numpy_reference(hs, mask, labels, W, bb, st, en, tr)

    em_np = ml_dtypes.float8_e4m3 if EM_FP8 else ml_dtypes.bfloat16
    expT, g4, ra, rb, id8 = _host_consts(tr)

    # X^T in matmul layout: [B, 128, HC*S], partition k holds H rows c*128+k
    if EM_FP8:
        xc = hs.astype(em_np)
    else:
        xc = hs.astype(em_np)
    xT = np.ascontiguousarray(
        xc.reshape(B, S, HC, 128).transpose(0, 3, 2, 1)
    ).reshape(B, 128, HC * S)
    ws = (W * WSCALE) if EM_FP8 else W
    wT = np.ascontiguousarray(
        ws.reshape(HC, 128, L).transpose(1, 0, 2)
    ).astype(em_np)                                   # [128, HC, L]
    if EM_FP8:
        wp = np.zeros((128, HC, 16), dtype=em_np)
        wp[:, :, :L] = wT
        wT = wp
    wT = wT.reshape(128, -1)

    nc = _get_nc()
    in_maps = []
    for k in range(NCORES):
        sl = slice(k * BPC, (k + 1) * BPC)
        in_maps.append(
            {
                "xT": xT[sl],
                "Wt": wT,
                "G4": g4,
                "Ra": ra,
                "Rb": rb,
                "Id8": id8,
            }
        )
    res = bass_utils.run_bass_kernel_spmd(nc, in_maps, list(range(NCORES)))
    _CACHE["last_results"] = res

    # ---- host combine (f64, tiny) ----
    expT64 = np.exp(tr.astype(np.float64))
    e_end = np.exp(en.astype(np.float64))
    st64 = st.astype(np.float64)
    bb64 = bb.astype(np.float64)
    en64 = en.astype(np.float64)
    tr64 = tr.astype(np.float64)
    total = 0.0
    for k in range(NCORES):
        r = res.results[k]
        em = r["em_out"].astype(np.float64)          # [BPC, 9, S]
        if EM_FP8:
            em = em / WSCALE
        Sf = r["S_out"].astype(np.float64).reshape(BPC, NCHUNK, L, L)
        mv = r["m_out"].astype(np.float64).reshape(BPC, NCHUNK, NNORM)
        for b in range(BPC):
            v = np.exp(em[b, :, 0] + st64 + bb64)    # v0
            logacc = -np.log(mv[b]).sum()            # undo applied scales
            for c in range(NCHUNK):
                v = v @ Sf[b, c]
                m = v.max()
                v /= m
                logacc += np.log(m)
            v = (v @ expT64) * np.exp(em[b, :, S - 1] + bb64)  # tail t = S-1
            total += np.log(v @ e_end) + logacc
        # numerator for this core's sequences (gold path score)
        lb = labels[k * BPC : (k + 1) * BPC]
        em_tag = np.take_along_axis(em, lb[:, None, :], axis=1)[:, 0, :]  # [BPC,S]
        total -= float(
            em_tag.sum()
            + st64[lb[:, 0]].sum()
            + en64[lb[:, -1]].sum()
            + tr64[lb[:, :-1], lb[:, 1:]].sum()
            + bb64[lb].sum()
        )
    return np.asarray(total, dtype=np.float32)
